# revision 18
# baseline (speedup 1.0000x reference)
"""Trainium2 Bass kernel for nn_DenseMoE (32-expert grouped-top-6 MoE + shared expert).

Strategy (expert-parallel across 8 NeuronCores):
  - Host: routing (bit-exact jax-on-CPU replica of the reference grouped top-k),
    per-expert token gather with per-slot capacity, weight repack/transpose to
    bf16 pre-tiled layouts, shared-expert sharded along its intermediate dim.
  - Device (SPMD, per core): 4 routed experts' SwiGLU on their gathered tokens
    + 1/8 of the shared expert's SwiGLU over all tokens. bf16 matmuls, f32 PSUM.
  - Host: weighted scatter-add of routed outputs + sum of shared partials.
"""

import os
import sys

_TRN = "/opt/trn_rl_repo"
if os.path.isdir(_TRN) and _TRN not in sys.path:
    sys.path.insert(0, _TRN)

import numpy as np
import ml_dtypes

import concourse.bass as bass
import concourse.tile as tile
from concourse import bacc, mybir
from concourse import bass_utils

BF16 = mybir.dt.bfloat16
F32 = mybir.dt.float32
NP_BF16 = ml_dtypes.bfloat16

HID = 2048
I_MOE = 1408
N_EXP = 32
N_GROUP = 8
TOPK_GROUP = 3
TOP_K = 6
T = 1024
SCALE = 1.0

N_CORES = 8
E_LOC = 4                 # routed experts per core
KT1 = HID // 128          # 16 contraction tiles for the up/gate matmul
PAIRS = I_MOE // 128      # 11 gate/up pairs
MT1 = 2 * PAIRS           # 22 m-tiles (gate/up interleaved): 5 blocks of 4 + 1 of 2
MB1 = 5                   # full m-blocks of 4 tiles (last half-block separate)
KT2 = PAIRS               # 11 contraction tiles for the down matmul
MT2 = HID // 128          # 16 output tiles
MB2 = 4                   # m2-blocks of 4 tiles
IS_BLK = 3                # shared-expert intermediate 128-blocks per core (22 -> 24 padded)

_BUILD_CACHE = {}
LAST_EXEC_NS = None


# ---------------------------------------------------------------- routing ----

def _routing(x, Wg):
    """Replicates reference grouped_topk on jax-CPU (bit-exact selection)."""
    import jax
    import jax.numpy as jnp

    cpu = jax.devices("cpu")[0]
    with jax.default_device(cpu):
        xj = jnp.asarray(x)
        wj = jnp.asarray(Wg)
        logits = xj @ wj.T
        scores = jax.nn.softmax(logits, axis=-1)
        n_tok, n_exp = scores.shape
        group_scores = scores.reshape(n_tok, N_GROUP, -1).max(axis=-1)
        _, group_idx = jax.lax.top_k(group_scores, TOPK_GROUP)
        group_mask = jnp.zeros_like(group_scores).at[
            jnp.arange(n_tok)[:, None], group_idx].set(1.0)
        score_mask = jnp.repeat(group_mask, n_exp // N_GROUP, axis=-1)
        tmp = jnp.where(score_mask > 0, scores, 0.0)
        topk_w, topk_ids = jax.lax.top_k(tmp, TOP_K)
        return np.asarray(topk_w), np.asarray(topk_ids)


# ---------------------------------------------------------------- device -----

def _build(caps):
    """Build + compile the per-core SPMD program. caps[j] = slot-j capacity."""
    key = tuple(caps)
    if key in _BUILD_CACHE:
        return _BUILD_CACHE[key]
    C0 = caps[0]

    nc = bacc.Bacc("TRN2", target_bir_lowering=False, debug=False,
                   num_devices=N_CORES)
    xg = nc.dram_tensor("xg", (E_LOC, 128, KT1, C0), BF16,
                        kind="ExternalInput").ap()
    xt = nc.dram_tensor("xt", (128, KT1, T), BF16, kind="ExternalInput").ap()
    wgu = nc.dram_tensor("wgu", (E_LOC, MB1, 128, KT1, 512), BF16,
                         kind="ExternalInput").ap()
    wgu5 = nc.dram_tensor("wgu5", (E_LOC, 128, KT1, 256), BF16,
                          kind="ExternalInput").ap()
    wd = nc.dram_tensor("wd", (E_LOC, MB2, 128, KT2, 512), BF16,
                        kind="ExternalInput").ap()
    wgs = nc.dram_tensor("wgs", (IS_BLK, 128, KT1, 256), BF16,
                         kind="ExternalInput").ap()
    wds = nc.dram_tensor("wds", (MB2, 128, IS_BLK, 512), BF16,
                         kind="ExternalInput").ap()
    og = nc.dram_tensor("og", (E_LOC, MB2, 128, 4, C0), BF16,
                        kind="ExternalOutput").ap()
    osh = nc.dram_tensor("osh", (2, MB2, 128, 4, 512), BF16,
                         kind="ExternalOutput").ap()

    SILU = mybir.ActivationFunctionType.Silu

    with tile.TileContext(nc) as tc:
        with (
            tc.tile_pool(name="xga", bufs=2) as xgpool,
            tc.tile_pool(name="xta", bufs=1) as xtpool,
            tc.tile_pool(name="w1", bufs=4) as w1pool,
            tc.tile_pool(name="w2", bufs=2) as w2pool,
            tc.tile_pool(name="wsh", bufs=3) as wshpool,
            tc.tile_pool(name="hp", bufs=22) as hpool,
            tc.tile_pool(name="hs", bufs=6) as hspool,
            tc.tile_pool(name="sil", bufs=2) as silupool,
            tc.tile_pool(name="ot", bufs=2) as opool,
            tc.tile_pool(name="ots", bufs=2) as ospool,
            tc.tile_pool(name="ps1", bufs=4, space="PSUM") as psum1,
            tc.tile_pool(name="ps2", bufs=4, space="PSUM") as psum2,
        ):
            # shared-expert tensors are needed only at the tail; their loads
            # are dripped between routed weight-block DMAs so they never
            # delay the critical weight stream (esp. not the kernel head).
            xta = xtpool.tile([128, KT1, T], BF16)
            wgs_tiles = []
            wds_tiles = []
            drip = []
            for kt in range(KT1):
                drip.append(("xt", kt))
            for p in range(IS_BLK):
                drip.append(("wgs", p))
            for b2 in range(MB2):
                drip.append(("wds", b2))
            drip_i = [0]
            drip_tick = [0]

            def drip_one():
                drip_tick[0] += 1
                if drip_tick[0] % 2 == 0 and drip_i[0] < len(drip) - 10:
                    return
                if drip_i[0] >= len(drip):
                    return
                kind, idx = drip[drip_i[0]]
                drip_i[0] += 1
                if kind == "xt":
                    nc.sync.dma_start(xta[:, idx, :], xt[:, idx, :])
                elif kind == "wgs":
                    wt = wshpool.tile([128, KT1, 256], BF16, name="wgs_t",
                                      tag="wgs", bufs=3)
                    nc.sync.dma_start(wt[:], wgs[idx])
                    wgs_tiles.append(wt)
                else:
                    wt = wshpool.tile([128, IS_BLK, 512], BF16, name="wds_t",
                                      tag="wds", bufs=4)
                    nc.sync.dma_start(wt[:], wds[idx])
                    wds_tiles.append(wt)

            # ---------------- routed experts ----------------
            xga_tiles = [None] * E_LOC
            xga_tiles[0] = xgpool.tile([128, KT1, caps[0]], BF16, name="xga",
                                       tag="xga")
            # split so the first matmuls wait on ~0.3MB, not the full set
            nc.sync.dma_start(xga_tiles[0][:, 0:2, :], xg[0][:, 0:2, :caps[0]])
            nc.sync.dma_start(xga_tiles[0][:, 2:KT1, :],
                              xg[0][:, 2:KT1, :caps[0]])

            w1_tiles = {}

            def fetch_w1(j, b, split=False):
                if (j, b) in w1_tiles:
                    return w1_tiles[(j, b)]
                if b < MB1:
                    w1 = w1pool.tile([128, KT1, 512], BF16, name="w1",
                                     tag="w1")
                    if split:
                        nc.sync.dma_start(w1[:, 0:2, :], wgu[j, b][:, 0:2, :])
                        nc.sync.dma_start(w1[:, 2:6, :], wgu[j, b][:, 2:6, :])
                        nc.sync.dma_start(w1[:, 6:11, :], wgu[j, b][:, 6:11, :])
                        nc.sync.dma_start(w1[:, 11:KT1, :],
                                          wgu[j, b][:, 11:KT1, :])
                    else:
                        nc.sync.dma_start(w1[:], wgu[j, b])
                else:
                    w1 = w1pool.tile([128, KT1, 256], BF16, name="w1",
                                     tag="w1")
                    nc.sync.dma_start(w1[:], wgu5[j])
                w1_tiles[(j, b)] = w1
                return w1

            fetch_w1(0, 0, split=True)

            def emit_l2_block(pj, b2, ph_tiles, pC):
                w2 = w2pool.tile([128, KT2, 512], BF16, name="w2", tag="w2")
                nc.sync.dma_start(w2[:], wd[pj, b2])
                drip_one()
                ps2 = [psum2.tile([128, pC], F32, name="ps2", tag="ps2")
                       for _ in range(4)]
                for kt2 in range(KT2):
                    for i in range(4):
                        nc.tensor.matmul(
                            ps2[i][:], w2[:, kt2, i * 128:(i + 1) * 128],
                            ph_tiles[kt2][:],
                            start=(kt2 == 0), stop=(kt2 == KT2 - 1))
                ot = opool.tile([128, 4, pC], BF16, name="ot", tag="ot")
                for i in range(4):
                    nc.any.tensor_copy(ot[:, i, :], ps2[i][:])
                nc.sync.dma_start(og[pj, b2][:, :, :pC], ot[:])

            # L2 of expert j-1 is interleaved into L1 of expert j: the PE
            # then always has ready work while the next w1 block streams in,
            # and the weight-DMA demand stays flat across the boundary.
            prev = None
            for j in range(E_LOC):
                C = caps[j]
                xga = xga_tiles[j]

                h_tiles = [None] * PAIRS
                for b in range(MB1 + 1):
                    n_m = 4 if b < MB1 else 2
                    w1 = fetch_w1(j, b)
                    if not (j == 0 and b == 0):
                        drip_one()
                    ps = [psum1.tile([128, C], F32, name="ps1", tag="ps1")
                          for _ in range(4)]
                    for kt in range(KT1):
                        for i in range(n_m):
                            nc.tensor.matmul(
                                ps[i][:], w1[:, kt, i * 128:(i + 1) * 128],
                                xga[:, kt, :],
                                start=(kt == 0), stop=(kt == KT1 - 1))
                    for pi in range(n_m // 2):
                        hidx = 2 * b + pi
                        sil = silupool.tile([128, C], F32, name="sil",
                                            tag="sil")
                        nc.scalar.activation(sil[:], ps[2 * pi][:], SILU)
                        ht = hpool.tile([128, C], BF16, name="ht", tag="ht")
                        nc.vector.tensor_mul(ht[:], sil[:], ps[2 * pi + 1][:])
                        h_tiles[hidx] = ht
                    w1_tiles.pop((j, b))
                    if prev is not None and 1 <= b <= 4:
                        emit_l2_block(prev[0], b - 1, prev[1], prev[2])
                    if b == 3 and j + 1 < E_LOC:
                        nxt = xgpool.tile([128, KT1, caps[j + 1]], BF16,
                                          name="xga", tag="xga")
                        nc.sync.dma_start(nxt[:], xg[j + 1][:, :, :caps[j + 1]])
                        xga_tiles[j + 1] = nxt
                prev = (j, h_tiles, C)

            # last expert's L2 (shared-expert weights are preloaded, so no
            # DMA spike follows)
            for b2 in range(MB2):
                emit_l2_block(prev[0], b2, prev[1], prev[2])

            # ---------------- shared expert (1/8 shard) ----------------
            for ch in range(2):
                tok = bass.ds(ch * 512, 512)
                hs_tiles = [None] * IS_BLK
                for p in range(IS_BLK):
                    w1 = wgs_tiles[p]
                    psg = psum1.tile([128, 512], F32, name="psg", tag="ps1")
                    psu = psum1.tile([128, 512], F32, name="psu", tag="ps1")
                    for kt in range(KT1):
                        nc.tensor.matmul(psg[:], w1[:, kt, 0:128],
                                         xta[:, kt, tok],
                                         start=(kt == 0), stop=(kt == KT1 - 1))
                        nc.tensor.matmul(psu[:], w1[:, kt, 128:256],
                                         xta[:, kt, tok],
                                         start=(kt == 0), stop=(kt == KT1 - 1))
                    sil = silupool.tile([128, 512], F32, name="sil", tag="sil")
                    nc.scalar.activation(sil[:], psg[:], SILU)
                    ht = hspool.tile([128, 512], BF16, name="hts", tag="hts")
                    nc.vector.tensor_mul(ht[:], sil[:], psu[:])
                    hs_tiles[p] = ht

                for b2 in range(MB2):
                    w2 = wds_tiles[b2]
                    ps2 = [psum2.tile([128, 512], F32, name="ps2", tag="ps2")
                           for _ in range(4)]
                    for kt2 in range(IS_BLK):
                        for i in range(4):
                            nc.tensor.matmul(
                                ps2[i][:], w2[:, kt2, i * 128:(i + 1) * 128],
                                hs_tiles[kt2][:],
                                start=(kt2 == 0), stop=(kt2 == IS_BLK - 1))
                    ot = ospool.tile([128, 4, 512], BF16, name="ots",
                                     tag="ots")
                    for i in range(4):
                        nc.any.tensor_copy(ot[:, i, :], ps2[i][:])
                    nc.gpsimd.dma_start(osh[ch, b2], ot[:])

    nc.compile()
    _BUILD_CACHE[key] = nc
    return nc


# ---------------------------------------------------------------- packing ----

def _pack_expert_weights(Wgu_e, Wd_e):
    """Pre-tile one routed expert's weights into the DMA-friendly layouts."""
    # up/gate weight: m-tiles gate/up interleaved; 5 blocks of 4 + 1 of 2.
    inter = np.empty((MT1, 128, HID), np.float32)
    inter[0:MT1:2] = Wgu_e[:I_MOE].reshape(PAIRS, 128, HID)
    inter[1:MT1:2] = Wgu_e[I_MOE:].reshape(PAIRS, 128, HID)
    # [m, c, h] -> [h, m*128]
    a = inter.transpose(2, 0, 1).reshape(HID, MT1 * 128)
    main = a[:, :MB1 * 512]                   # [HID, 5*512]
    main = main.reshape(KT1, 128, MB1, 512).transpose(2, 1, 0, 3)
    w1 = np.ascontiguousarray(main).astype(NP_BF16)
    tail = a[:, MB1 * 512:]                   # [HID, 256]
    tail = tail.reshape(KT1, 128, 256).transpose(1, 0, 2)
    w15 = np.ascontiguousarray(tail).astype(NP_BF16)
    # down weight -> [MB2, 128, KT2, 512]
    b = Wd_e.T.reshape(KT2, 128, MB2, 512).transpose(2, 1, 0, 3)
    w2 = np.ascontiguousarray(b).astype(NP_BF16)
    return w1, w15, w2


def _prepare(inputs):
    x = np.asarray(inputs["x"], np.float32)
    Wg = np.asarray(inputs["Wg"], np.float32)
    Wgu = np.asarray(inputs["Wgu"], np.float32)
    Wd = np.asarray(inputs["Wd"], np.float32)
    Wgu_s = np.asarray(inputs["Wgu_s"], np.float32)
    Wd_s = np.asarray(inputs["Wd_s"], np.float32)

    topk_w, topk_ids = _routing(x, Wg)

    # token lists per expert
    idx_per_e = []
    w_per_e = []
    for e in range(N_EXP):
        tok, kpos = np.nonzero(topk_ids == e)
        idx_per_e.append(tok.astype(np.int64))
        w_per_e.append(topk_w[tok, kpos].astype(np.float32))
    loads = np.array([len(i) for i in idx_per_e])

    # assign experts to (core, slot): sort desc by load; slot j gets ranks
    # 8j..8j+7 across cores, so per-slot capacity = its max load (rounded).
    order = np.argsort(-loads, kind="stable")
    assign = np.zeros((N_CORES, E_LOC), np.int64)
    for j in range(E_LOC):
        for c in range(N_CORES):
            assign[c, j] = order[8 * j + c]
    caps = [int(max(8, -(-int(loads[order[8 * j]]) // 8) * 8))
            for j in range(E_LOC)]
    C0 = caps[0]

    xT = np.ascontiguousarray(x.T)             # [HID, T] f32

    in_maps = []
    meta = []
    xt_arr = np.ascontiguousarray(
        xT.reshape(KT1, 128, T).transpose(1, 0, 2)).astype(NP_BF16)

    I_S = Wd_s.shape[1] // 2 * 2  # 2816
    n_sblk = I_S // 128           # 22

    for c in range(N_CORES):
        m = {}
        # routed token gathers: xg [E_LOC, 128, KT1, C0]
        xg_arr = np.zeros((E_LOC, HID, C0), np.float32)
        for j in range(E_LOC):
            e = assign[c, j]
            L = loads[e]
            xg_arr[j, :, :L] = xT[:, idx_per_e[e]]
        xg_arr = xg_arr.reshape(E_LOC, KT1, 128, C0).transpose(0, 2, 1, 3)
        m["xg"] = np.ascontiguousarray(xg_arr).astype(NP_BF16)
        m["xt"] = xt_arr

        w1_all = np.empty((E_LOC, MB1, 128, KT1, 512), NP_BF16)
        w15_all = np.empty((E_LOC, 128, KT1, 256), NP_BF16)
        w2_all = np.empty((E_LOC, MB2, 128, KT2, 512), NP_BF16)
        for j in range(E_LOC):
            e = assign[c, j]
            w1_all[j], w15_all[j], w2_all[j] = _pack_expert_weights(
                Wgu[e], Wd[e])
        m["wgu"] = w1_all
        m["wgu5"] = w15_all
        m["wd"] = w2_all

        # shared shard: blocks 3c, 3c+1, 3c+2 of the 22 (pad w/ zeros)
        wgs_arr = np.zeros((HID, IS_BLK, 256), np.float32)
        wds_arr = np.zeros((IS_BLK, 128, HID), np.float32)
        for p in range(IS_BLK):
            g = 3 * c + p
            if g < n_sblk:
                wgs_arr[:, p, :128] = Wgu_s[128 * g:128 * (g + 1)].T
                wgs_arr[:, p, 128:] = Wgu_s[I_S + 128 * g:I_S + 128 * (g + 1)].T
                wds_arr[p] = Wd_s[:, 128 * g:128 * (g + 1)].T
        wgs_t = wgs_arr.reshape(KT1, 128, IS_BLK, 256).transpose(2, 1, 0, 3)
        m["wgs"] = np.ascontiguousarray(wgs_t).astype(NP_BF16)
        wds_t = wds_arr.reshape(IS_BLK, 128, MB2, 512).transpose(2, 1, 0, 3)
        m["wds"] = np.ascontiguousarray(wds_t).astype(NP_BF16)

        in_maps.append(m)
        meta.append([(assign[c, j], idx_per_e[assign[c, j]],
                      w_per_e[assign[c, j]]) for j in range(E_LOC)])

    return in_maps, meta, caps


# ---------------------------------------------------------------- entry ------

def kernel(x, Wg, Wgu, Wd, Wgu_s, Wd_s, _trace=False):
    global LAST_EXEC_NS
    inputs = dict(x=x, Wg=Wg, Wgu=Wgu, Wd=Wd, Wgu_s=Wgu_s, Wd_s=Wd_s)
    in_maps, meta, caps = _prepare(inputs)
    C0 = caps[0]
    nc = _build(caps)

    res = bass_utils.run_bass_kernel_spmd(
        nc, in_maps, core_ids=list(range(N_CORES)), trace=_trace)
    LAST_EXEC_NS = res.exec_time_ns

    out = np.zeros((T, HID), np.float32)
    for c in range(N_CORES):
        osh = res.results[c]["osh"].astype(np.float32)
        shared_T = osh.transpose(1, 3, 2, 0, 4).reshape(HID, T)
        out += shared_T.T
        og = res.results[c]["og"].astype(np.float32)
        for j in range(E_LOC):
            e, idx, wts = meta[c][j]
            if len(idx) == 0:
                continue
            blk = og[j].transpose(0, 2, 1, 3).reshape(HID, C0)
            out[idx] += wts[:, None] * blk[:, :len(idx)].T * SCALE
    return out


# revision 19
# speedup vs baseline: 1.0911x; 1.0911x over previous
"""Trainium2 Bass kernel for nn_DenseMoE (32-expert grouped-top-6 MoE + shared expert).

Strategy (expert-parallel across 8 NeuronCores):
  - Host: routing (bit-exact jax-on-CPU replica of the reference grouped top-k),
    per-expert token gather with per-slot capacity, weight repack/transpose to
    bf16 pre-tiled layouts, shared-expert sharded along its intermediate dim.
  - Device (SPMD, per core): 4 routed experts' SwiGLU on their gathered tokens
    + 1/8 of the shared expert's SwiGLU over all tokens. bf16 matmuls, f32 PSUM.
  - Host: weighted scatter-add of routed outputs + sum of shared partials.
"""

import os
import sys

_TRN = "/opt/trn_rl_repo"
if os.path.isdir(_TRN) and _TRN not in sys.path:
    sys.path.insert(0, _TRN)

import numpy as np
import ml_dtypes

import concourse.bass as bass
import concourse.tile as tile
from concourse import bacc, mybir
from concourse import bass_utils

BF16 = mybir.dt.bfloat16
F32 = mybir.dt.float32
NP_BF16 = ml_dtypes.bfloat16

HID = 2048
I_MOE = 1408
N_EXP = 32
N_GROUP = 8
TOPK_GROUP = 3
TOP_K = 6
T = 1024
SCALE = 1.0

N_CORES = 8
E_LOC = 4                 # routed experts per core
KT1 = HID // 128          # 16 contraction tiles for the up/gate matmul
PAIRS = I_MOE // 128      # 11 gate/up pairs
MT1 = 2 * PAIRS           # 22 m-tiles (gate/up interleaved): 5 blocks of 4 + 1 of 2
MB1 = 5                   # full m-blocks of 4 tiles (last half-block separate)
KT2 = PAIRS               # 11 contraction tiles for the down matmul
MT2 = HID // 128          # 16 output tiles
MB2 = 4                   # m2-blocks of 4 tiles
IS_BLK = 3                # shared-expert intermediate 128-blocks per core (22 -> 24 padded)

_BUILD_CACHE = {}
LAST_EXEC_NS = None


# ---------------------------------------------------------------- routing ----

def _routing(x, Wg):
    """Replicates reference grouped_topk on jax-CPU (bit-exact selection)."""
    import jax
    import jax.numpy as jnp

    cpu = jax.devices("cpu")[0]
    with jax.default_device(cpu):
        xj = jnp.asarray(x)
        wj = jnp.asarray(Wg)
        logits = xj @ wj.T
        scores = jax.nn.softmax(logits, axis=-1)
        n_tok, n_exp = scores.shape
        group_scores = scores.reshape(n_tok, N_GROUP, -1).max(axis=-1)
        _, group_idx = jax.lax.top_k(group_scores, TOPK_GROUP)
        group_mask = jnp.zeros_like(group_scores).at[
            jnp.arange(n_tok)[:, None], group_idx].set(1.0)
        score_mask = jnp.repeat(group_mask, n_exp // N_GROUP, axis=-1)
        tmp = jnp.where(score_mask > 0, scores, 0.0)
        topk_w, topk_ids = jax.lax.top_k(tmp, TOP_K)
        return np.asarray(topk_w), np.asarray(topk_ids)


# ---------------------------------------------------------------- device -----

def _build(caps):
    """Build + compile the per-core SPMD program. caps[j] = slot-j capacity."""
    key = tuple(caps)
    if key in _BUILD_CACHE:
        return _BUILD_CACHE[key]
    C0 = caps[0]

    nc = bacc.Bacc("TRN2", target_bir_lowering=False, debug=False,
                   num_devices=N_CORES)
    xg = nc.dram_tensor("xg", (E_LOC, 128, KT1, C0), BF16,
                        kind="ExternalInput").ap()
    xt = nc.dram_tensor("xt", (128, KT1, T), BF16, kind="ExternalInput").ap()
    wgu = nc.dram_tensor("wgu", (E_LOC, MB1, 128, KT1, 512), BF16,
                         kind="ExternalInput").ap()
    wgu5 = nc.dram_tensor("wgu5", (E_LOC, 128, KT1, 256), BF16,
                          kind="ExternalInput").ap()
    wd = nc.dram_tensor("wd", (E_LOC, MB2, 128, KT2, 512), BF16,
                        kind="ExternalInput").ap()
    wgs = nc.dram_tensor("wgs", (IS_BLK, 128, KT1, 256), BF16,
                         kind="ExternalInput").ap()
    wds = nc.dram_tensor("wds", (MB2, 128, IS_BLK, 512), BF16,
                         kind="ExternalInput").ap()
    og = nc.dram_tensor("og", (E_LOC, MB2, 128, 4, C0), BF16,
                        kind="ExternalOutput").ap()
    osh = nc.dram_tensor("osh", (2, MB2, 128, 4, 512), BF16,
                         kind="ExternalOutput").ap()

    SILU = mybir.ActivationFunctionType.Silu

    with tile.TileContext(nc) as tc:
        with (
            tc.tile_pool(name="xga", bufs=2) as xgpool,
            tc.tile_pool(name="xta", bufs=1) as xtpool,
            tc.tile_pool(name="w1", bufs=4) as w1pool,
            tc.tile_pool(name="w2", bufs=2) as w2pool,
            tc.tile_pool(name="wsh", bufs=3) as wshpool,
            tc.tile_pool(name="hp", bufs=22) as hpool,
            tc.tile_pool(name="hs", bufs=6) as hspool,
            tc.tile_pool(name="sil", bufs=2) as silupool,
            tc.tile_pool(name="ot", bufs=2) as opool,
            tc.tile_pool(name="ots", bufs=2) as ospool,
            tc.tile_pool(name="ps1", bufs=4, space="PSUM") as psum1,
            tc.tile_pool(name="ps2", bufs=4, space="PSUM") as psum2,
        ):
            # shared-expert tensors are needed only at the tail; their loads
            # are dripped between routed weight-block DMAs so they never
            # delay the critical weight stream (esp. not the kernel head).
            xta = xtpool.tile([128, KT1, T], BF16)
            wgs_tiles = []
            wds_tiles = []
            drip = []
            for kt in range(KT1):
                drip.append(("xt", kt))
            for p in range(IS_BLK):
                drip.append(("wgs", p))
            for b2 in range(MB2):
                drip.append(("wds", b2))
            drip_i = [0]
            drip_tick = [0]

            def drip_one():
                drip_tick[0] += 1
                if drip_tick[0] % 2 == 0 and drip_i[0] < len(drip) - 10:
                    return
                if drip_i[0] >= len(drip):
                    return
                kind, idx = drip[drip_i[0]]
                drip_i[0] += 1
                if kind == "xt":
                    nc.sync.dma_start(xta[:, idx, :], xt[:, idx, :])
                elif kind == "wgs":
                    wt = wshpool.tile([128, KT1, 256], BF16, name="wgs_t",
                                      tag="wgs", bufs=3)
                    nc.sync.dma_start(wt[:], wgs[idx])
                    wgs_tiles.append(wt)
                else:
                    wt = wshpool.tile([128, IS_BLK, 512], BF16, name="wds_t",
                                      tag="wds", bufs=4)
                    nc.sync.dma_start(wt[:], wds[idx])
                    wds_tiles.append(wt)

            # ---------------- routed experts ----------------
            xga_tiles = [None] * E_LOC
            xga_tiles[0] = xgpool.tile([128, KT1, caps[0]], BF16, name="xga",
                                       tag="xga")
            # split so the first matmuls wait on ~0.3MB, not the full set
            nc.sync.dma_start(xga_tiles[0][:, 0:2, :], xg[0][:, 0:2, :caps[0]])
            nc.sync.dma_start(xga_tiles[0][:, 2:KT1, :],
                              xg[0][:, 2:KT1, :caps[0]])

            w1_tiles = {}

            def fetch_w1(j, b, split=False):
                if (j, b) in w1_tiles:
                    return w1_tiles[(j, b)]
                if b < MB1:
                    w1 = w1pool.tile([128, KT1, 512], BF16, name="w1",
                                     tag="w1")
                    if split:
                        nc.sync.dma_start(w1[:, 0:2, :], wgu[j, b][:, 0:2, :])
                        nc.sync.dma_start(w1[:, 2:6, :], wgu[j, b][:, 2:6, :])
                        nc.sync.dma_start(w1[:, 6:11, :], wgu[j, b][:, 6:11, :])
                        nc.sync.dma_start(w1[:, 11:KT1, :],
                                          wgu[j, b][:, 11:KT1, :])
                    else:
                        nc.sync.dma_start(w1[:], wgu[j, b])
                else:
                    w1 = w1pool.tile([128, KT1, 256], BF16, name="w1",
                                     tag="w1")
                    nc.sync.dma_start(w1[:], wgu5[j])
                w1_tiles[(j, b)] = w1
                return w1

            fetch_w1(0, 0, split=True)

            def emit_l2_block(pj, b2, ph_tiles, pC):
                w2 = w2pool.tile([128, KT2, 512], BF16, name="w2", tag="w2")
                nc.scalar.dma_start(w2[:], wd[pj, b2])
                drip_one()
                ps2 = [psum2.tile([128, pC], F32, name="ps2", tag="ps2")
                       for _ in range(4)]
                for kt2 in range(KT2):
                    for i in range(4):
                        nc.tensor.matmul(
                            ps2[i][:], w2[:, kt2, i * 128:(i + 1) * 128],
                            ph_tiles[kt2][:],
                            start=(kt2 == 0), stop=(kt2 == KT2 - 1))
                ot = opool.tile([128, 4, pC], BF16, name="ot", tag="ot")
                for i in range(4):
                    nc.any.tensor_copy(ot[:, i, :], ps2[i][:])
                nc.scalar.dma_start(og[pj, b2][:, :, :pC], ot[:])

            # L2 of expert j-1 is interleaved into L1 of expert j: the PE
            # then always has ready work while the next w1 block streams in,
            # and the weight-DMA demand stays flat across the boundary.
            prev = None
            for j in range(E_LOC):
                C = caps[j]
                xga = xga_tiles[j]

                h_tiles = [None] * PAIRS
                for b in range(MB1 + 1):
                    n_m = 4 if b < MB1 else 2
                    w1 = fetch_w1(j, b)
                    if not (j == 0 and b == 0):
                        drip_one()
                    ps = [psum1.tile([128, C], F32, name="ps1", tag="ps1")
                          for _ in range(4)]
                    for kt in range(KT1):
                        for i in range(n_m):
                            nc.tensor.matmul(
                                ps[i][:], w1[:, kt, i * 128:(i + 1) * 128],
                                xga[:, kt, :],
                                start=(kt == 0), stop=(kt == KT1 - 1))
                    for pi in range(n_m // 2):
                        hidx = 2 * b + pi
                        sil = silupool.tile([128, C], F32, name="sil",
                                            tag="sil")
                        nc.scalar.activation(sil[:], ps[2 * pi][:], SILU)
                        ht = hpool.tile([128, C], BF16, name="ht", tag="ht")
                        nc.vector.tensor_mul(ht[:], sil[:], ps[2 * pi + 1][:])
                        h_tiles[hidx] = ht
                    w1_tiles.pop((j, b))
                    if prev is not None and 1 <= b <= 4:
                        emit_l2_block(prev[0], b - 1, prev[1], prev[2])
                    if b == 3 and j + 1 < E_LOC:
                        nxt = xgpool.tile([128, KT1, caps[j + 1]], BF16,
                                          name="xga", tag="xga")
                        nc.sync.dma_start(nxt[:], xg[j + 1][:, :, :caps[j + 1]])
                        xga_tiles[j + 1] = nxt
                prev = (j, h_tiles, C)

            # last expert's L2 (shared-expert weights are preloaded, so no
            # DMA spike follows)
            for b2 in range(MB2):
                emit_l2_block(prev[0], b2, prev[1], prev[2])

            # ---------------- shared expert (1/8 shard) ----------------
            for ch in range(2):
                tok = bass.ds(ch * 512, 512)
                hs_tiles = [None] * IS_BLK
                for p in range(IS_BLK):
                    w1 = wgs_tiles[p]
                    psg = psum1.tile([128, 512], F32, name="psg", tag="ps1")
                    psu = psum1.tile([128, 512], F32, name="psu", tag="ps1")
                    for kt in range(KT1):
                        nc.tensor.matmul(psg[:], w1[:, kt, 0:128],
                                         xta[:, kt, tok],
                                         start=(kt == 0), stop=(kt == KT1 - 1))
                        nc.tensor.matmul(psu[:], w1[:, kt, 128:256],
                                         xta[:, kt, tok],
                                         start=(kt == 0), stop=(kt == KT1 - 1))
                    sil = silupool.tile([128, 512], F32, name="sil", tag="sil")
                    nc.scalar.activation(sil[:], psg[:], SILU)
                    ht = hspool.tile([128, 512], BF16, name="hts", tag="hts")
                    nc.vector.tensor_mul(ht[:], sil[:], psu[:])
                    hs_tiles[p] = ht

                for b2 in range(MB2):
                    w2 = wds_tiles[b2]
                    ps2 = [psum2.tile([128, 512], F32, name="ps2", tag="ps2")
                           for _ in range(4)]
                    for kt2 in range(IS_BLK):
                        for i in range(4):
                            nc.tensor.matmul(
                                ps2[i][:], w2[:, kt2, i * 128:(i + 1) * 128],
                                hs_tiles[kt2][:],
                                start=(kt2 == 0), stop=(kt2 == IS_BLK - 1))
                    ot = ospool.tile([128, 4, 512], BF16, name="ots",
                                     tag="ots")
                    for i in range(4):
                        nc.any.tensor_copy(ot[:, i, :], ps2[i][:])
                    nc.gpsimd.dma_start(osh[ch, b2], ot[:])

    nc.compile()
    _BUILD_CACHE[key] = nc
    return nc


# ---------------------------------------------------------------- packing ----

def _pack_expert_weights(Wgu_e, Wd_e):
    """Pre-tile one routed expert's weights into the DMA-friendly layouts."""
    # up/gate weight: m-tiles gate/up interleaved; 5 blocks of 4 + 1 of 2.
    inter = np.empty((MT1, 128, HID), np.float32)
    inter[0:MT1:2] = Wgu_e[:I_MOE].reshape(PAIRS, 128, HID)
    inter[1:MT1:2] = Wgu_e[I_MOE:].reshape(PAIRS, 128, HID)
    # [m, c, h] -> [h, m*128]
    a = inter.transpose(2, 0, 1).reshape(HID, MT1 * 128)
    main = a[:, :MB1 * 512]                   # [HID, 5*512]
    main = main.reshape(KT1, 128, MB1, 512).transpose(2, 1, 0, 3)
    w1 = np.ascontiguousarray(main).astype(NP_BF16)
    tail = a[:, MB1 * 512:]                   # [HID, 256]
    tail = tail.reshape(KT1, 128, 256).transpose(1, 0, 2)
    w15 = np.ascontiguousarray(tail).astype(NP_BF16)
    # down weight -> [MB2, 128, KT2, 512]
    b = Wd_e.T.reshape(KT2, 128, MB2, 512).transpose(2, 1, 0, 3)
    w2 = np.ascontiguousarray(b).astype(NP_BF16)
    return w1, w15, w2


def _prepare(inputs):
    x = np.asarray(inputs["x"], np.float32)
    Wg = np.asarray(inputs["Wg"], np.float32)
    Wgu = np.asarray(inputs["Wgu"], np.float32)
    Wd = np.asarray(inputs["Wd"], np.float32)
    Wgu_s = np.asarray(inputs["Wgu_s"], np.float32)
    Wd_s = np.asarray(inputs["Wd_s"], np.float32)

    topk_w, topk_ids = _routing(x, Wg)

    # token lists per expert
    idx_per_e = []
    w_per_e = []
    for e in range(N_EXP):
        tok, kpos = np.nonzero(topk_ids == e)
        idx_per_e.append(tok.astype(np.int64))
        w_per_e.append(topk_w[tok, kpos].astype(np.float32))
    loads = np.array([len(i) for i in idx_per_e])

    # assign experts to (core, slot): sort desc by load; slot j gets ranks
    # 8j..8j+7 across cores, so per-slot capacity = its max load (rounded).
    order = np.argsort(-loads, kind="stable")
    assign = np.zeros((N_CORES, E_LOC), np.int64)
    for j in range(E_LOC):
        for c in range(N_CORES):
            assign[c, j] = order[8 * j + c]
    caps = [int(max(8, -(-int(loads[order[8 * j]]) // 8) * 8))
            for j in range(E_LOC)]
    C0 = caps[0]

    xT = np.ascontiguousarray(x.T)             # [HID, T] f32

    in_maps = []
    meta = []
    xt_arr = np.ascontiguousarray(
        xT.reshape(KT1, 128, T).transpose(1, 0, 2)).astype(NP_BF16)

    I_S = Wd_s.shape[1] // 2 * 2  # 2816
    n_sblk = I_S // 128           # 22

    for c in range(N_CORES):
        m = {}
        # routed token gathers: xg [E_LOC, 128, KT1, C0]
        xg_arr = np.zeros((E_LOC, HID, C0), np.float32)
        for j in range(E_LOC):
            e = assign[c, j]
            L = loads[e]
            xg_arr[j, :, :L] = xT[:, idx_per_e[e]]
        xg_arr = xg_arr.reshape(E_LOC, KT1, 128, C0).transpose(0, 2, 1, 3)
        m["xg"] = np.ascontiguousarray(xg_arr).astype(NP_BF16)
        m["xt"] = xt_arr

        w1_all = np.empty((E_LOC, MB1, 128, KT1, 512), NP_BF16)
        w15_all = np.empty((E_LOC, 128, KT1, 256), NP_BF16)
        w2_all = np.empty((E_LOC, MB2, 128, KT2, 512), NP_BF16)
        for j in range(E_LOC):
            e = assign[c, j]
            w1_all[j], w15_all[j], w2_all[j] = _pack_expert_weights(
                Wgu[e], Wd[e])
        m["wgu"] = w1_all
        m["wgu5"] = w15_all
        m["wd"] = w2_all

        # shared shard: blocks 3c, 3c+1, 3c+2 of the 22 (pad w/ zeros)
        wgs_arr = np.zeros((HID, IS_BLK, 256), np.float32)
        wds_arr = np.zeros((IS_BLK, 128, HID), np.float32)
        for p in range(IS_BLK):
            g = 3 * c + p
            if g < n_sblk:
                wgs_arr[:, p, :128] = Wgu_s[128 * g:128 * (g + 1)].T
                wgs_arr[:, p, 128:] = Wgu_s[I_S + 128 * g:I_S + 128 * (g + 1)].T
                wds_arr[p] = Wd_s[:, 128 * g:128 * (g + 1)].T
        wgs_t = wgs_arr.reshape(KT1, 128, IS_BLK, 256).transpose(2, 1, 0, 3)
        m["wgs"] = np.ascontiguousarray(wgs_t).astype(NP_BF16)
        wds_t = wds_arr.reshape(IS_BLK, 128, MB2, 512).transpose(2, 1, 0, 3)
        m["wds"] = np.ascontiguousarray(wds_t).astype(NP_BF16)

        in_maps.append(m)
        meta.append([(assign[c, j], idx_per_e[assign[c, j]],
                      w_per_e[assign[c, j]]) for j in range(E_LOC)])

    return in_maps, meta, caps


# ---------------------------------------------------------------- entry ------

def kernel(x, Wg, Wgu, Wd, Wgu_s, Wd_s, _trace=False):
    global LAST_EXEC_NS
    inputs = dict(x=x, Wg=Wg, Wgu=Wgu, Wd=Wd, Wgu_s=Wgu_s, Wd_s=Wd_s)
    in_maps, meta, caps = _prepare(inputs)
    C0 = caps[0]
    nc = _build(caps)

    res = bass_utils.run_bass_kernel_spmd(
        nc, in_maps, core_ids=list(range(N_CORES)), trace=_trace)
    LAST_EXEC_NS = res.exec_time_ns

    out = np.zeros((T, HID), np.float32)
    for c in range(N_CORES):
        osh = res.results[c]["osh"].astype(np.float32)
        shared_T = osh.transpose(1, 3, 2, 0, 4).reshape(HID, T)
        out += shared_T.T
        og = res.results[c]["og"].astype(np.float32)
        for j in range(E_LOC):
            e, idx, wts = meta[c][j]
            if len(idx) == 0:
                continue
            blk = og[j].transpose(0, 2, 1, 3).reshape(HID, C0)
            out[idx] += wts[:, None] * blk[:, :len(idx)].T * SCALE
    return out


# revision 20
# speedup vs baseline: 1.1933x; 1.0937x over previous
"""Trainium2 Bass kernel for nn_DenseMoE (32-expert grouped-top-6 MoE + shared expert).

Strategy (expert-parallel across 8 NeuronCores):
  - Host: routing (bit-exact jax-on-CPU replica of the reference grouped top-k),
    per-expert token gather with per-slot capacity, weight repack/transpose to
    bf16 pre-tiled layouts, shared-expert sharded along its intermediate dim.
  - Device (SPMD, per core): 4 routed experts' SwiGLU on their gathered tokens
    + 1/8 of the shared expert's SwiGLU over all tokens. bf16 matmuls, f32 PSUM.
  - Host: weighted scatter-add of routed outputs + sum of shared partials.
"""

import os
import sys

_TRN = "/opt/trn_rl_repo"
if os.path.isdir(_TRN) and _TRN not in sys.path:
    sys.path.insert(0, _TRN)

import numpy as np
import ml_dtypes

import concourse.bass as bass
import concourse.tile as tile
from concourse import bacc, mybir
from concourse import bass_utils

BF16 = mybir.dt.bfloat16
F32 = mybir.dt.float32
NP_BF16 = ml_dtypes.bfloat16

HID = 2048
I_MOE = 1408
N_EXP = 32
N_GROUP = 8
TOPK_GROUP = 3
TOP_K = 6
T = 1024
SCALE = 1.0

N_CORES = 8
E_LOC = 4                 # routed experts per core
KT1 = HID // 128          # 16 contraction tiles for the up/gate matmul
PAIRS = I_MOE // 128      # 11 gate/up pairs
MT1 = 2 * PAIRS           # 22 m-tiles (gate/up interleaved): 5 blocks of 4 + 1 of 2
MB1 = 5                   # full m-blocks of 4 tiles (last half-block separate)
KT2 = PAIRS               # 11 contraction tiles for the down matmul
MT2 = HID // 128          # 16 output tiles
MB2 = 4                   # m2-blocks of 4 tiles
IS_BLK = 3                # shared-expert intermediate 128-blocks per core (22 -> 24 padded)

_BUILD_CACHE = {}
LAST_EXEC_NS = None


# ---------------------------------------------------------------- routing ----

def _routing(x, Wg):
    """Replicates reference grouped_topk on jax-CPU (bit-exact selection)."""
    import jax
    import jax.numpy as jnp

    cpu = jax.devices("cpu")[0]
    with jax.default_device(cpu):
        xj = jnp.asarray(x)
        wj = jnp.asarray(Wg)
        logits = xj @ wj.T
        scores = jax.nn.softmax(logits, axis=-1)
        n_tok, n_exp = scores.shape
        group_scores = scores.reshape(n_tok, N_GROUP, -1).max(axis=-1)
        _, group_idx = jax.lax.top_k(group_scores, TOPK_GROUP)
        group_mask = jnp.zeros_like(group_scores).at[
            jnp.arange(n_tok)[:, None], group_idx].set(1.0)
        score_mask = jnp.repeat(group_mask, n_exp // N_GROUP, axis=-1)
        tmp = jnp.where(score_mask > 0, scores, 0.0)
        topk_w, topk_ids = jax.lax.top_k(tmp, TOP_K)
        return np.asarray(topk_w), np.asarray(topk_ids)


# ---------------------------------------------------------------- device -----

def _build(caps):
    """Build + compile the per-core SPMD program. caps[j] = slot-j capacity."""
    key = tuple(caps)
    if key in _BUILD_CACHE:
        return _BUILD_CACHE[key]
    C0 = caps[0]

    nc = bacc.Bacc("TRN2", target_bir_lowering=False, debug=False,
                   num_devices=N_CORES)
    xg = nc.dram_tensor("xg", (E_LOC, 128, KT1, C0), BF16,
                        kind="ExternalInput").ap()
    xt = nc.dram_tensor("xt", (128, KT1, T), BF16, kind="ExternalInput").ap()
    wgu = nc.dram_tensor("wgu", (E_LOC, MB1, 128, KT1, 512), BF16,
                         kind="ExternalInput").ap()
    wgu5 = nc.dram_tensor("wgu5", (E_LOC, 128, KT1, 256), BF16,
                          kind="ExternalInput").ap()
    wd = nc.dram_tensor("wd", (E_LOC, MB2, 128, KT2, 512), BF16,
                        kind="ExternalInput").ap()
    wgs = nc.dram_tensor("wgs", (IS_BLK, 128, KT1, 256), BF16,
                         kind="ExternalInput").ap()
    wds = nc.dram_tensor("wds", (MB2, 128, IS_BLK, 512), BF16,
                         kind="ExternalInput").ap()
    og = nc.dram_tensor("og", (E_LOC, MB2, 128, 4, C0), BF16,
                        kind="ExternalOutput").ap()
    osh = nc.dram_tensor("osh", (2, MB2, 128, 4, 512), BF16,
                         kind="ExternalOutput").ap()

    SILU = mybir.ActivationFunctionType.Silu

    with tile.TileContext(nc) as tc:
        with (
            tc.tile_pool(name="xga", bufs=2) as xgpool,
            tc.tile_pool(name="xta", bufs=1) as xtpool,
            tc.tile_pool(name="w1", bufs=4) as w1pool,
            tc.tile_pool(name="w2", bufs=2) as w2pool,
            tc.tile_pool(name="wsh", bufs=3) as wshpool,
            tc.tile_pool(name="hp", bufs=22) as hpool,
            tc.tile_pool(name="hs", bufs=6) as hspool,
            tc.tile_pool(name="sil", bufs=2) as silupool,
            tc.tile_pool(name="ot", bufs=2) as opool,
            tc.tile_pool(name="ots", bufs=2) as ospool,
            tc.tile_pool(name="ps1", bufs=4, space="PSUM") as psum1,
            tc.tile_pool(name="ps2", bufs=4, space="PSUM") as psum2,
        ):
            # shared-expert tensors are needed only at the tail; their loads
            # are dripped between routed weight-block DMAs so they never
            # delay the critical weight stream (esp. not the kernel head).
            xta = xtpool.tile([128, KT1, T], BF16)
            wgs_tiles = []
            wds_tiles = []
            drip = [("xt", kt) for kt in range(0, KT1, 2)]
            drip += [("wgs", p) for p in range(IS_BLK)]
            drip += [("wds", b2) for b2 in range(MB2)]
            drip_i = [0]

            def drip_one():
                if drip_i[0] >= len(drip):
                    return
                kind, idx = drip[drip_i[0]]
                drip_i[0] += 1
                if kind == "xt":
                    nc.sync.dma_start(xta[:, idx:idx + 2, :],
                                      xt[:, idx:idx + 2, :])
                elif kind == "wgs":
                    wt = wshpool.tile([128, KT1, 256], BF16, name="wgs_t",
                                      tag="wgs", bufs=3)
                    nc.sync.dma_start(wt[:], wgs[idx])
                    wgs_tiles.append(wt)
                else:
                    wt = wshpool.tile([128, IS_BLK, 512], BF16, name="wds_t",
                                      tag="wds", bufs=4)
                    nc.sync.dma_start(wt[:], wds[idx])
                    wds_tiles.append(wt)

            # ---------------- shared-expert work pieces ----------------
            # The shared expert costs PE time but ~zero DMA (weights + xT are
            # preloaded/dripped), so its pieces act as relief valves wherever
            # the routed weight stream would otherwise starve the PE.
            hs_tiles = {0: [None] * IS_BLK, 1: [None] * IS_BLK}

            def emit_shared_l1(ch, p):
                tok = bass.ds(ch * 512, 512)
                w1 = wgs_tiles[p]
                psg = psum1.tile([128, 512], F32, name="psg", tag="ps1")
                psu = psum1.tile([128, 512], F32, name="psu", tag="ps1")
                for kt in range(KT1):
                    nc.tensor.matmul(psg[:], w1[:, kt, 0:128],
                                     xta[:, kt, tok],
                                     start=(kt == 0), stop=(kt == KT1 - 1))
                    nc.tensor.matmul(psu[:], w1[:, kt, 128:256],
                                     xta[:, kt, tok],
                                     start=(kt == 0), stop=(kt == KT1 - 1))
                sil = silupool.tile([128, 512], F32, name="sil", tag="sil")
                nc.scalar.activation(sil[:], psg[:], SILU)
                ht = hspool.tile([128, 512], BF16, name="hts", tag="hts")
                nc.vector.tensor_mul(ht[:], sil[:], psu[:])
                hs_tiles[ch][p] = ht

            def emit_shared_l2(ch, b2):
                w2 = wds_tiles[b2]
                ps2 = [psum2.tile([128, 512], F32, name="ps2", tag="ps2")
                       for _ in range(4)]
                for kt2 in range(IS_BLK):
                    for i in range(4):
                        nc.tensor.matmul(
                            ps2[i][:], w2[:, kt2, i * 128:(i + 1) * 128],
                            hs_tiles[ch][kt2][:],
                            start=(kt2 == 0), stop=(kt2 == IS_BLK - 1))
                ot = ospool.tile([128, 4, 512], BF16, name="ots", tag="ots")
                for i in range(4):
                    nc.any.tensor_copy(ot[:, i, :], ps2[i][:])
                nc.sync.dma_start(osh[ch, b2], ot[:])

            shared_q = []
            for ch in range(2):
                shared_q += [("l1", ch, p) for p in range(IS_BLK)]
                shared_q += [("l2", ch, b2) for b2 in range(MB2)]
            sq_i = [0]

            def emit_shared(n):
                for _ in range(n):
                    if sq_i[0] >= len(shared_q):
                        return
                    kind, ch, idx = shared_q[sq_i[0]]
                    sq_i[0] += 1
                    if kind == "l1":
                        emit_shared_l1(ch, idx)
                    else:
                        emit_shared_l2(ch, idx)

            # ---------------- routed experts ----------------
            xga_tiles = [None] * E_LOC
            xga_tiles[0] = xgpool.tile([128, KT1, caps[0]], BF16, name="xga",
                                       tag="xga")
            # split so the first matmuls wait on ~0.3MB, not the full set
            nc.sync.dma_start(xga_tiles[0][:, 0:2, :], xg[0][:, 0:2, :caps[0]])
            nc.sync.dma_start(xga_tiles[0][:, 2:KT1, :],
                              xg[0][:, 2:KT1, :caps[0]])

            w1_tiles = {}

            def fetch_w1(j, b, split=False):
                if (j, b) in w1_tiles:
                    return w1_tiles[(j, b)]
                if b < MB1:
                    w1 = w1pool.tile([128, KT1, 512], BF16, name="w1",
                                     tag="w1")
                    if split:
                        nc.sync.dma_start(w1[:, 0:2, :], wgu[j, b][:, 0:2, :])
                        nc.sync.dma_start(w1[:, 2:6, :], wgu[j, b][:, 2:6, :])
                        nc.sync.dma_start(w1[:, 6:11, :], wgu[j, b][:, 6:11, :])
                        nc.sync.dma_start(w1[:, 11:KT1, :],
                                          wgu[j, b][:, 11:KT1, :])
                    else:
                        nc.sync.dma_start(w1[:], wgu[j, b])
                else:
                    w1 = w1pool.tile([128, KT1, 256], BF16, name="w1",
                                     tag="w1")
                    nc.sync.dma_start(w1[:], wgu5[j])
                w1_tiles[(j, b)] = w1
                return w1

            fetch_w1(0, 0, split=True)

            for j in range(E_LOC):
                C = caps[j]
                xga = xga_tiles[j]

                h_tiles = [None] * PAIRS
                for b in range(MB1 + 1):
                    n_m = 4 if b < MB1 else 2
                    w1 = fetch_w1(j, b)
                    if not (j == 0 and b == 0):
                        drip_one()
                    ps = [psum1.tile([128, C], F32, name="ps1", tag="ps1")
                          for _ in range(4)]
                    for kt in range(KT1):
                        for i in range(n_m):
                            nc.tensor.matmul(
                                ps[i][:], w1[:, kt, i * 128:(i + 1) * 128],
                                xga[:, kt, :],
                                start=(kt == 0), stop=(kt == KT1 - 1))
                    for pi in range(n_m // 2):
                        hidx = 2 * b + pi
                        sil = silupool.tile([128, C], F32, name="sil",
                                            tag="sil")
                        nc.scalar.activation(sil[:], ps[2 * pi][:], SILU)
                        ht = hpool.tile([128, C], BF16, name="ht", tag="ht")
                        nc.vector.tensor_mul(ht[:], sil[:], ps[2 * pi + 1][:])
                        h_tiles[hidx] = ht
                    w1_tiles.pop((j, b))
                    if j >= 1 and b in (0, 2):
                        emit_shared(2 if b == 0 else 1)

                # next expert's tokens, issued before the w2 slot-waits can
                # head-of-line-block them on the sync queue
                if j + 1 < E_LOC:
                    nxt = xgpool.tile([128, KT1, caps[j + 1]], BF16,
                                      name="xga", tag="xga")
                    nc.sync.dma_start(nxt[:], xg[j + 1][:, :, :caps[j + 1]])
                    xga_tiles[j + 1] = nxt

                for b2 in range(MB2):
                    w2 = w2pool.tile([128, KT2, 512], BF16, name="w2",
                                     tag="w2")
                    nc.sync.dma_start(w2[:], wd[j, b2])
                    drip_one()
                    ps2 = [psum2.tile([128, C], F32, name="ps2", tag="ps2")
                           for _ in range(4)]
                    for kt2 in range(KT2):
                        for i in range(4):
                            nc.tensor.matmul(
                                ps2[i][:], w2[:, kt2, i * 128:(i + 1) * 128],
                                h_tiles[kt2][:],
                                start=(kt2 == 0), stop=(kt2 == KT2 - 1))
                    ot = opool.tile([128, 4, C], BF16, name="ot", tag="ot")
                    for i in range(4):
                        nc.any.tensor_copy(ot[:, i, :], ps2[i][:])
                    nc.sync.dma_start(og[j, b2][:, :, :C], ot[:])
                    if j >= 2:
                        emit_shared(1)

            # remaining shared pieces
            emit_shared(len(shared_q))

    nc.compile()
    _BUILD_CACHE[key] = nc
    return nc


# ---------------------------------------------------------------- packing ----

def _pack_expert_weights(Wgu_e, Wd_e):
    """Pre-tile one routed expert's weights into the DMA-friendly layouts."""
    # up/gate weight: m-tiles gate/up interleaved; 5 blocks of 4 + 1 of 2.
    inter = np.empty((MT1, 128, HID), np.float32)
    inter[0:MT1:2] = Wgu_e[:I_MOE].reshape(PAIRS, 128, HID)
    inter[1:MT1:2] = Wgu_e[I_MOE:].reshape(PAIRS, 128, HID)
    # [m, c, h] -> [h, m*128]
    a = inter.transpose(2, 0, 1).reshape(HID, MT1 * 128)
    main = a[:, :MB1 * 512]                   # [HID, 5*512]
    main = main.reshape(KT1, 128, MB1, 512).transpose(2, 1, 0, 3)
    w1 = np.ascontiguousarray(main).astype(NP_BF16)
    tail = a[:, MB1 * 512:]                   # [HID, 256]
    tail = tail.reshape(KT1, 128, 256).transpose(1, 0, 2)
    w15 = np.ascontiguousarray(tail).astype(NP_BF16)
    # down weight -> [MB2, 128, KT2, 512]
    b = Wd_e.T.reshape(KT2, 128, MB2, 512).transpose(2, 1, 0, 3)
    w2 = np.ascontiguousarray(b).astype(NP_BF16)
    return w1, w15, w2


def _prepare(inputs):
    x = np.asarray(inputs["x"], np.float32)
    Wg = np.asarray(inputs["Wg"], np.float32)
    Wgu = np.asarray(inputs["Wgu"], np.float32)
    Wd = np.asarray(inputs["Wd"], np.float32)
    Wgu_s = np.asarray(inputs["Wgu_s"], np.float32)
    Wd_s = np.asarray(inputs["Wd_s"], np.float32)

    topk_w, topk_ids = _routing(x, Wg)

    # token lists per expert
    idx_per_e = []
    w_per_e = []
    for e in range(N_EXP):
        tok, kpos = np.nonzero(topk_ids == e)
        idx_per_e.append(tok.astype(np.int64))
        w_per_e.append(topk_w[tok, kpos].astype(np.float32))
    loads = np.array([len(i) for i in idx_per_e])

    # assign experts to (core, slot): sort desc by load; slot j gets ranks
    # 8j..8j+7 across cores, so per-slot capacity = its max load (rounded).
    order = np.argsort(-loads, kind="stable")
    assign = np.zeros((N_CORES, E_LOC), np.int64)
    for j in range(E_LOC):
        for c in range(N_CORES):
            assign[c, j] = order[8 * j + c]
    caps = [int(max(8, -(-int(loads[order[8 * j]]) // 8) * 8))
            for j in range(E_LOC)]
    C0 = caps[0]

    xT = np.ascontiguousarray(x.T)             # [HID, T] f32

    in_maps = []
    meta = []
    xt_arr = np.ascontiguousarray(
        xT.reshape(KT1, 128, T).transpose(1, 0, 2)).astype(NP_BF16)

    I_S = Wd_s.shape[1] // 2 * 2  # 2816
    n_sblk = I_S // 128           # 22

    for c in range(N_CORES):
        m = {}
        # routed token gathers: xg [E_LOC, 128, KT1, C0]
        xg_arr = np.zeros((E_LOC, HID, C0), np.float32)
        for j in range(E_LOC):
            e = assign[c, j]
            L = loads[e]
            xg_arr[j, :, :L] = xT[:, idx_per_e[e]]
        xg_arr = xg_arr.reshape(E_LOC, KT1, 128, C0).transpose(0, 2, 1, 3)
        m["xg"] = np.ascontiguousarray(xg_arr).astype(NP_BF16)
        m["xt"] = xt_arr

        w1_all = np.empty((E_LOC, MB1, 128, KT1, 512), NP_BF16)
        w15_all = np.empty((E_LOC, 128, KT1, 256), NP_BF16)
        w2_all = np.empty((E_LOC, MB2, 128, KT2, 512), NP_BF16)
        for j in range(E_LOC):
            e = assign[c, j]
            w1_all[j], w15_all[j], w2_all[j] = _pack_expert_weights(
                Wgu[e], Wd[e])
        m["wgu"] = w1_all
        m["wgu5"] = w15_all
        m["wd"] = w2_all

        # shared shard: blocks 3c, 3c+1, 3c+2 of the 22 (pad w/ zeros)
        wgs_arr = np.zeros((HID, IS_BLK, 256), np.float32)
        wds_arr = np.zeros((IS_BLK, 128, HID), np.float32)
        for p in range(IS_BLK):
            g = 3 * c + p
            if g < n_sblk:
                wgs_arr[:, p, :128] = Wgu_s[128 * g:128 * (g + 1)].T
                wgs_arr[:, p, 128:] = Wgu_s[I_S + 128 * g:I_S + 128 * (g + 1)].T
                wds_arr[p] = Wd_s[:, 128 * g:128 * (g + 1)].T
        wgs_t = wgs_arr.reshape(KT1, 128, IS_BLK, 256).transpose(2, 1, 0, 3)
        m["wgs"] = np.ascontiguousarray(wgs_t).astype(NP_BF16)
        wds_t = wds_arr.reshape(IS_BLK, 128, MB2, 512).transpose(2, 1, 0, 3)
        m["wds"] = np.ascontiguousarray(wds_t).astype(NP_BF16)

        in_maps.append(m)
        meta.append([(assign[c, j], idx_per_e[assign[c, j]],
                      w_per_e[assign[c, j]]) for j in range(E_LOC)])

    return in_maps, meta, caps


# ---------------------------------------------------------------- entry ------

def kernel(x, Wg, Wgu, Wd, Wgu_s, Wd_s, _trace=False):
    global LAST_EXEC_NS
    inputs = dict(x=x, Wg=Wg, Wgu=Wgu, Wd=Wd, Wgu_s=Wgu_s, Wd_s=Wd_s)
    in_maps, meta, caps = _prepare(inputs)
    C0 = caps[0]
    nc = _build(caps)

    res = bass_utils.run_bass_kernel_spmd(
        nc, in_maps, core_ids=list(range(N_CORES)), trace=_trace)
    LAST_EXEC_NS = res.exec_time_ns

    out = np.zeros((T, HID), np.float32)
    for c in range(N_CORES):
        osh = res.results[c]["osh"].astype(np.float32)
        shared_T = osh.transpose(1, 3, 2, 0, 4).reshape(HID, T)
        out += shared_T.T
        og = res.results[c]["og"].astype(np.float32)
        for j in range(E_LOC):
            e, idx, wts = meta[c][j]
            if len(idx) == 0:
                continue
            blk = og[j].transpose(0, 2, 1, 3).reshape(HID, C0)
            out[idx] += wts[:, None] * blk[:, :len(idx)].T * SCALE
    return out


# revision 21
# speedup vs baseline: 1.1976x; 1.0036x over previous
"""Trainium2 Bass kernel for nn_DenseMoE (32-expert grouped-top-6 MoE + shared expert).

Strategy (expert-parallel across 8 NeuronCores):
  - Host: routing (bit-exact jax-on-CPU replica of the reference grouped top-k),
    per-expert token gather with per-slot capacity, weight repack/transpose to
    bf16 pre-tiled layouts, shared-expert sharded along its intermediate dim.
  - Device (SPMD, per core): 4 routed experts' SwiGLU on their gathered tokens
    + 1/8 of the shared expert's SwiGLU over all tokens. bf16 matmuls, f32 PSUM.
  - Host: weighted scatter-add of routed outputs + sum of shared partials.
"""

import os
import sys

_TRN = "/opt/trn_rl_repo"
if os.path.isdir(_TRN) and _TRN not in sys.path:
    sys.path.insert(0, _TRN)

import numpy as np
import ml_dtypes

import concourse.bass as bass
import concourse.tile as tile
from concourse import bacc, mybir
from concourse import bass_utils

BF16 = mybir.dt.bfloat16
F32 = mybir.dt.float32
NP_BF16 = ml_dtypes.bfloat16

HID = 2048
I_MOE = 1408
N_EXP = 32
N_GROUP = 8
TOPK_GROUP = 3
TOP_K = 6
T = 1024
SCALE = 1.0

N_CORES = 8
E_LOC = 4                 # routed experts per core
KT1 = HID // 128          # 16 contraction tiles for the up/gate matmul
PAIRS = I_MOE // 128      # 11 gate/up pairs
MT1 = 2 * PAIRS           # 22 m-tiles (gate/up interleaved): 5 blocks of 4 + 1 of 2
MB1 = 5                   # full m-blocks of 4 tiles (last half-block separate)
KT2 = PAIRS               # 11 contraction tiles for the down matmul
MT2 = HID // 128          # 16 output tiles
MB2 = 4                   # m2-blocks of 4 tiles
IS_BLK = 3                # shared-expert intermediate 128-blocks per core (22 -> 24 padded)

_BUILD_CACHE = {}
LAST_EXEC_NS = None


# ---------------------------------------------------------------- routing ----

def _routing(x, Wg):
    """Replicates reference grouped_topk on jax-CPU (bit-exact selection)."""
    import jax
    import jax.numpy as jnp

    cpu = jax.devices("cpu")[0]
    with jax.default_device(cpu):
        xj = jnp.asarray(x)
        wj = jnp.asarray(Wg)
        logits = xj @ wj.T
        scores = jax.nn.softmax(logits, axis=-1)
        n_tok, n_exp = scores.shape
        group_scores = scores.reshape(n_tok, N_GROUP, -1).max(axis=-1)
        _, group_idx = jax.lax.top_k(group_scores, TOPK_GROUP)
        group_mask = jnp.zeros_like(group_scores).at[
            jnp.arange(n_tok)[:, None], group_idx].set(1.0)
        score_mask = jnp.repeat(group_mask, n_exp // N_GROUP, axis=-1)
        tmp = jnp.where(score_mask > 0, scores, 0.0)
        topk_w, topk_ids = jax.lax.top_k(tmp, TOP_K)
        return np.asarray(topk_w), np.asarray(topk_ids)


# ---------------------------------------------------------------- device -----

def _build(caps):
    """Build + compile the per-core SPMD program. caps[j] = slot-j capacity."""
    key = tuple(caps)
    if key in _BUILD_CACHE:
        return _BUILD_CACHE[key]
    C0 = caps[0]

    nc = bacc.Bacc("TRN2", target_bir_lowering=False, debug=False,
                   num_devices=N_CORES)
    xg = nc.dram_tensor("xg", (E_LOC, 128, KT1, C0), BF16,
                        kind="ExternalInput").ap()
    xt = nc.dram_tensor("xt", (128, KT1, T), BF16, kind="ExternalInput").ap()
    wgu = nc.dram_tensor("wgu", (E_LOC, MB1, 128, KT1, 512), BF16,
                         kind="ExternalInput").ap()
    wgu5 = nc.dram_tensor("wgu5", (E_LOC, 128, KT1, 256), BF16,
                          kind="ExternalInput").ap()
    wd = nc.dram_tensor("wd", (E_LOC, MB2, 128, KT2, 512), BF16,
                        kind="ExternalInput").ap()
    wgs = nc.dram_tensor("wgs", (IS_BLK, 128, KT1, 256), BF16,
                         kind="ExternalInput").ap()
    wds = nc.dram_tensor("wds", (MB2, 128, IS_BLK, 512), BF16,
                         kind="ExternalInput").ap()
    og = nc.dram_tensor("og", (E_LOC, MB2, 128, 4, C0), BF16,
                        kind="ExternalOutput").ap()
    osh = nc.dram_tensor("osh", (2, MB2, 128, 4, 512), BF16,
                         kind="ExternalOutput").ap()

    SILU = mybir.ActivationFunctionType.Silu

    with tile.TileContext(nc) as tc:
        with (
            tc.tile_pool(name="xga", bufs=2) as xgpool,
            tc.tile_pool(name="xta", bufs=1) as xtpool,
            tc.tile_pool(name="w1", bufs=4) as w1pool,
            tc.tile_pool(name="w2", bufs=2) as w2pool,
            tc.tile_pool(name="wsh", bufs=3) as wshpool,
            tc.tile_pool(name="hp", bufs=22) as hpool,
            tc.tile_pool(name="hs", bufs=6) as hspool,
            tc.tile_pool(name="sil", bufs=2) as silupool,
            tc.tile_pool(name="ot", bufs=2) as opool,
            tc.tile_pool(name="ots", bufs=2) as ospool,
            tc.tile_pool(name="ps1", bufs=4, space="PSUM") as psum1,
            tc.tile_pool(name="ps2", bufs=4, space="PSUM") as psum2,
        ):
            # shared-expert tensors are needed only at the tail; their loads
            # are dripped between routed weight-block DMAs so they never
            # delay the critical weight stream (esp. not the kernel head).
            xta = xtpool.tile([128, KT1, T], BF16)
            wgs_tiles = []
            wds_tiles = []
            drip = [("xt0", kt) for kt in range(0, KT1, 2)]
            drip += [("wgs", p) for p in range(IS_BLK)]
            drip += [("xt1", kt) for kt in range(0, KT1, 2)]
            drip += [("wds", b2) for b2 in range(MB2)]
            drip_i = [0]

            def drip_one():
                if drip_i[0] >= len(drip):
                    return
                kind, idx = drip[drip_i[0]]
                drip_i[0] += 1
                if kind == "xt0":
                    nc.sync.dma_start(xta[:, idx:idx + 2, 0:512],
                                      xt[:, idx:idx + 2, 0:512])
                elif kind == "xt1":
                    nc.sync.dma_start(xta[:, idx:idx + 2, 512:T],
                                      xt[:, idx:idx + 2, 512:T])
                elif kind == "wgs":
                    wt = wshpool.tile([128, KT1, 256], BF16, name="wgs_t",
                                      tag="wgs", bufs=3)
                    nc.sync.dma_start(wt[:], wgs[idx])
                    wgs_tiles.append(wt)
                else:
                    wt = wshpool.tile([128, IS_BLK, 512], BF16, name="wds_t",
                                      tag="wds", bufs=4)
                    nc.sync.dma_start(wt[:], wds[idx])
                    wds_tiles.append(wt)

            # ---------------- shared-expert work pieces ----------------
            # The shared expert costs PE time but ~zero DMA (weights + xT are
            # preloaded/dripped), so its pieces act as relief valves wherever
            # the routed weight stream would otherwise starve the PE.
            hs_tiles = {0: [None] * IS_BLK, 1: [None] * IS_BLK}

            def emit_shared_l1(ch, p):
                tok = bass.ds(ch * 512, 512)
                w1 = wgs_tiles[p]
                psg = psum1.tile([128, 512], F32, name="psg", tag="ps1")
                psu = psum1.tile([128, 512], F32, name="psu", tag="ps1")
                for kt in range(KT1):
                    nc.tensor.matmul(psg[:], w1[:, kt, 0:128],
                                     xta[:, kt, tok],
                                     start=(kt == 0), stop=(kt == KT1 - 1))
                    nc.tensor.matmul(psu[:], w1[:, kt, 128:256],
                                     xta[:, kt, tok],
                                     start=(kt == 0), stop=(kt == KT1 - 1))
                sil = silupool.tile([128, 512], F32, name="sil", tag="sil")
                nc.scalar.activation(sil[:], psg[:], SILU)
                ht = hspool.tile([128, 512], BF16, name="hts", tag="hts")
                nc.vector.tensor_mul(ht[:], sil[:], psu[:])
                hs_tiles[ch][p] = ht

            def emit_shared_l2(ch, b2):
                w2 = wds_tiles[b2]
                ps2 = [psum2.tile([128, 512], F32, name="ps2", tag="ps2")
                       for _ in range(4)]
                for kt2 in range(IS_BLK):
                    for i in range(4):
                        nc.tensor.matmul(
                            ps2[i][:], w2[:, kt2, i * 128:(i + 1) * 128],
                            hs_tiles[ch][kt2][:],
                            start=(kt2 == 0), stop=(kt2 == IS_BLK - 1))
                ot = ospool.tile([128, 4, 512], BF16, name="ots", tag="ots")
                for i in range(4):
                    nc.any.tensor_copy(ot[:, i, :], ps2[i][:])
                nc.sync.dma_start(osh[ch, b2], ot[:])

            shared_q = []
            for ch in range(2):
                shared_q += [("l1", ch, p) for p in range(IS_BLK)]
                shared_q += [("l2", ch, b2) for b2 in range(MB2)]
            sq_i = [0]

            def emit_shared(n):
                for _ in range(n):
                    if sq_i[0] >= len(shared_q):
                        return
                    kind, ch, idx = shared_q[sq_i[0]]
                    sq_i[0] += 1
                    if kind == "l1":
                        emit_shared_l1(ch, idx)
                    else:
                        emit_shared_l2(ch, idx)

            # ---------------- routed experts ----------------
            xga_tiles = [None] * E_LOC
            xga_tiles[0] = xgpool.tile([128, KT1, caps[0]], BF16, name="xga",
                                       tag="xga")
            # split so the first matmuls wait on ~0.3MB, not the full set
            nc.sync.dma_start(xga_tiles[0][:, 0:2, :], xg[0][:, 0:2, :caps[0]])
            nc.sync.dma_start(xga_tiles[0][:, 2:KT1, :],
                              xg[0][:, 2:KT1, :caps[0]])

            w1_tiles = {}

            def fetch_w1(j, b, split=False):
                if (j, b) in w1_tiles:
                    return w1_tiles[(j, b)]
                if b < MB1:
                    w1 = w1pool.tile([128, KT1, 512], BF16, name="w1",
                                     tag="w1")
                    if split:
                        nc.sync.dma_start(w1[:, 0:2, :], wgu[j, b][:, 0:2, :])
                        nc.sync.dma_start(w1[:, 2:6, :], wgu[j, b][:, 2:6, :])
                        nc.sync.dma_start(w1[:, 6:11, :], wgu[j, b][:, 6:11, :])
                        nc.sync.dma_start(w1[:, 11:KT1, :],
                                          wgu[j, b][:, 11:KT1, :])
                    else:
                        nc.sync.dma_start(w1[:], wgu[j, b])
                else:
                    w1 = w1pool.tile([128, KT1, 256], BF16, name="w1",
                                     tag="w1")
                    nc.sync.dma_start(w1[:], wgu5[j])
                w1_tiles[(j, b)] = w1
                return w1

            fetch_w1(0, 0, split=True)

            for j in range(E_LOC):
                C = caps[j]
                xga = xga_tiles[j]

                h_tiles = [None] * PAIRS
                for b in range(MB1 + 1):
                    n_m = 4 if b < MB1 else 2
                    w1 = fetch_w1(j, b)
                    if not (j == 0 and b == 0):
                        drip_one()
                    ps = [psum1.tile([128, C], F32, name="ps1", tag="ps1")
                          for _ in range(4)]
                    for kt in range(KT1):
                        for i in range(n_m):
                            nc.tensor.matmul(
                                ps[i][:], w1[:, kt, i * 128:(i + 1) * 128],
                                xga[:, kt, :],
                                start=(kt == 0), stop=(kt == KT1 - 1))
                    for pi in range(n_m // 2):
                        hidx = 2 * b + pi
                        sil = silupool.tile([128, C], F32, name="sil",
                                            tag="sil")
                        nc.scalar.activation(sil[:], ps[2 * pi][:], SILU)
                        ht = hpool.tile([128, C], BF16, name="ht", tag="ht")
                        nc.vector.tensor_mul(ht[:], sil[:], ps[2 * pi + 1][:])
                        h_tiles[hidx] = ht
                    w1_tiles.pop((j, b))
                    if j >= 1 and b in (1, 3):
                        emit_shared(2 if b == 1 else 1)

                # next expert's tokens, issued before the w2 slot-waits can
                # head-of-line-block them on the sync queue
                if j + 1 < E_LOC:
                    nxt = xgpool.tile([128, KT1, caps[j + 1]], BF16,
                                      name="xga", tag="xga")
                    nc.sync.dma_start(nxt[:], xg[j + 1][:, :, :caps[j + 1]])
                    xga_tiles[j + 1] = nxt

                for b2 in range(MB2):
                    w2 = w2pool.tile([128, KT2, 512], BF16, name="w2",
                                     tag="w2")
                    nc.sync.dma_start(w2[:], wd[j, b2])
                    drip_one()
                    ps2 = [psum2.tile([128, C], F32, name="ps2", tag="ps2")
                           for _ in range(4)]
                    for kt2 in range(KT2):
                        for i in range(4):
                            nc.tensor.matmul(
                                ps2[i][:], w2[:, kt2, i * 128:(i + 1) * 128],
                                h_tiles[kt2][:],
                                start=(kt2 == 0), stop=(kt2 == KT2 - 1))
                    ot = opool.tile([128, 4, C], BF16, name="ot", tag="ot")
                    for i in range(4):
                        nc.any.tensor_copy(ot[:, i, :], ps2[i][:])
                    nc.sync.dma_start(og[j, b2][:, :, :C], ot[:])
                    if j >= 2:
                        emit_shared(1)

            # remaining shared pieces
            emit_shared(len(shared_q))

    nc.compile()
    _BUILD_CACHE[key] = nc
    return nc


# ---------------------------------------------------------------- packing ----

def _pack_expert_weights(Wgu_e, Wd_e):
    """Pre-tile one routed expert's weights into the DMA-friendly layouts."""
    # up/gate weight: m-tiles gate/up interleaved; 5 blocks of 4 + 1 of 2.
    inter = np.empty((MT1, 128, HID), np.float32)
    inter[0:MT1:2] = Wgu_e[:I_MOE].reshape(PAIRS, 128, HID)
    inter[1:MT1:2] = Wgu_e[I_MOE:].reshape(PAIRS, 128, HID)
    # [m, c, h] -> [h, m*128]
    a = inter.transpose(2, 0, 1).reshape(HID, MT1 * 128)
    main = a[:, :MB1 * 512]                   # [HID, 5*512]
    main = main.reshape(KT1, 128, MB1, 512).transpose(2, 1, 0, 3)
    w1 = np.ascontiguousarray(main).astype(NP_BF16)
    tail = a[:, MB1 * 512:]                   # [HID, 256]
    tail = tail.reshape(KT1, 128, 256).transpose(1, 0, 2)
    w15 = np.ascontiguousarray(tail).astype(NP_BF16)
    # down weight -> [MB2, 128, KT2, 512]
    b = Wd_e.T.reshape(KT2, 128, MB2, 512).transpose(2, 1, 0, 3)
    w2 = np.ascontiguousarray(b).astype(NP_BF16)
    return w1, w15, w2


def _prepare(inputs):
    x = np.asarray(inputs["x"], np.float32)
    Wg = np.asarray(inputs["Wg"], np.float32)
    Wgu = np.asarray(inputs["Wgu"], np.float32)
    Wd = np.asarray(inputs["Wd"], np.float32)
    Wgu_s = np.asarray(inputs["Wgu_s"], np.float32)
    Wd_s = np.asarray(inputs["Wd_s"], np.float32)

    topk_w, topk_ids = _routing(x, Wg)

    # token lists per expert
    idx_per_e = []
    w_per_e = []
    for e in range(N_EXP):
        tok, kpos = np.nonzero(topk_ids == e)
        idx_per_e.append(tok.astype(np.int64))
        w_per_e.append(topk_w[tok, kpos].astype(np.float32))
    loads = np.array([len(i) for i in idx_per_e])

    # assign experts to (core, slot): sort desc by load; slot j gets ranks
    # 8j..8j+7 across cores, so per-slot capacity = its max load (rounded).
    order = np.argsort(-loads, kind="stable")
    assign = np.zeros((N_CORES, E_LOC), np.int64)
    for j in range(E_LOC):
        for c in range(N_CORES):
            assign[c, j] = order[8 * j + c]
    caps = [int(max(8, -(-int(loads[order[8 * j]]) // 8) * 8))
            for j in range(E_LOC)]
    C0 = caps[0]

    xT = np.ascontiguousarray(x.T)             # [HID, T] f32

    in_maps = []
    meta = []
    xt_arr = np.ascontiguousarray(
        xT.reshape(KT1, 128, T).transpose(1, 0, 2)).astype(NP_BF16)

    I_S = Wd_s.shape[1] // 2 * 2  # 2816
    n_sblk = I_S // 128           # 22

    for c in range(N_CORES):
        m = {}
        # routed token gathers: xg [E_LOC, 128, KT1, C0]
        xg_arr = np.zeros((E_LOC, HID, C0), np.float32)
        for j in range(E_LOC):
            e = assign[c, j]
            L = loads[e]
            xg_arr[j, :, :L] = xT[:, idx_per_e[e]]
        xg_arr = xg_arr.reshape(E_LOC, KT1, 128, C0).transpose(0, 2, 1, 3)
        m["xg"] = np.ascontiguousarray(xg_arr).astype(NP_BF16)
        m["xt"] = xt_arr

        w1_all = np.empty((E_LOC, MB1, 128, KT1, 512), NP_BF16)
        w15_all = np.empty((E_LOC, 128, KT1, 256), NP_BF16)
        w2_all = np.empty((E_LOC, MB2, 128, KT2, 512), NP_BF16)
        for j in range(E_LOC):
            e = assign[c, j]
            w1_all[j], w15_all[j], w2_all[j] = _pack_expert_weights(
                Wgu[e], Wd[e])
        m["wgu"] = w1_all
        m["wgu5"] = w15_all
        m["wd"] = w2_all

        # shared shard: blocks 3c, 3c+1, 3c+2 of the 22 (pad w/ zeros)
        wgs_arr = np.zeros((HID, IS_BLK, 256), np.float32)
        wds_arr = np.zeros((IS_BLK, 128, HID), np.float32)
        for p in range(IS_BLK):
            g = 3 * c + p
            if g < n_sblk:
                wgs_arr[:, p, :128] = Wgu_s[128 * g:128 * (g + 1)].T
                wgs_arr[:, p, 128:] = Wgu_s[I_S + 128 * g:I_S + 128 * (g + 1)].T
                wds_arr[p] = Wd_s[:, 128 * g:128 * (g + 1)].T
        wgs_t = wgs_arr.reshape(KT1, 128, IS_BLK, 256).transpose(2, 1, 0, 3)
        m["wgs"] = np.ascontiguousarray(wgs_t).astype(NP_BF16)
        wds_t = wds_arr.reshape(IS_BLK, 128, MB2, 512).transpose(2, 1, 0, 3)
        m["wds"] = np.ascontiguousarray(wds_t).astype(NP_BF16)

        in_maps.append(m)
        meta.append([(assign[c, j], idx_per_e[assign[c, j]],
                      w_per_e[assign[c, j]]) for j in range(E_LOC)])

    return in_maps, meta, caps


# ---------------------------------------------------------------- entry ------

def kernel(x, Wg, Wgu, Wd, Wgu_s, Wd_s, _trace=False):
    global LAST_EXEC_NS
    inputs = dict(x=x, Wg=Wg, Wgu=Wgu, Wd=Wd, Wgu_s=Wgu_s, Wd_s=Wd_s)
    in_maps, meta, caps = _prepare(inputs)
    C0 = caps[0]
    nc = _build(caps)

    res = bass_utils.run_bass_kernel_spmd(
        nc, in_maps, core_ids=list(range(N_CORES)), trace=_trace)
    LAST_EXEC_NS = res.exec_time_ns

    out = np.zeros((T, HID), np.float32)
    for c in range(N_CORES):
        osh = res.results[c]["osh"].astype(np.float32)
        shared_T = osh.transpose(1, 3, 2, 0, 4).reshape(HID, T)
        out += shared_T.T
        og = res.results[c]["og"].astype(np.float32)
        for j in range(E_LOC):
            e, idx, wts = meta[c][j]
            if len(idx) == 0:
                continue
            blk = og[j].transpose(0, 2, 1, 3).reshape(HID, C0)
            out[idx] += wts[:, None] * blk[:, :len(idx)].T * SCALE
    return out
